# revision 5
# baseline (speedup 1.0000x reference)
"""Depthwise cross-correlation (per-sample dynamic kernel) on 8 Trainium2 cores.

reference: out[b,i,j,c] = sum_{di,dj} search[b,i+di,j+dj,c] * template[b,di,dj,c]
  search [64,31,31,256] f32, template [64,7,7,256] f32 -> out [64,25,25,256] f32

Strategy (pure data parallel, 8 samples/core, no collectives), all-bf16 compute:
- 49 taps split across three engine paths, balanced so PE/ACT/DVE finish
  together:
  * PE taps: diag(t_k) matmuls accumulated in PSUM (fp32), bf16 operands at
    1 col/cycle. Output rows split 13+12 so each accumulation target fits one
    PSUM bank.
  * ACT-paired taps: ScalarE per-channel multiply (Copy activation with
    per-partition scale) into a product buffer, VectorE tensor_tensor add
    (bf16 2x mode, both halves fused in one 4D instruction) into acc.
  * DVE-solo taps: single-pass FMA on VectorE.
- acc is folded into PSUM by one extra PE matmul with an identity stationary
  (accumulate), delayed one sample so PE never stalls on DVE.
- ScalarE evacuates the final PSUM to SBUF fp32; DMA out.
- Search rows are padded 31->32 cols so bf16 windows stay 4B-aligned (even dj
  for any DVE-strided read).
- A post-pass splits multi-wait instructions (walrus allows one sync-wait
  per instruction) into single-wait NoOp carriers.
"""
import sys

sys.path.insert(0, "/opt/trn_rl_repo")

import numpy as np
import ml_dtypes
import concourse.bass as bass
import concourse.mybir as mybir
import concourse.tile as tile
from concourse.bass_utils import run_bass_kernel_spmd

B = 64
X, K, OUT = 31, 7, 25
XP = 32                      # padded row length
CH = 256
C = 128                      # channels per half (partition dim)
N_CORES = 8
BPC = B // N_CORES           # samples per core
R0, R1 = 13, 12              # output row split (325 / 300 psum cols)

# ---- tap assignment (tune) -------------------------------------------------
ALL_TAPS = [(di, dj) for di in range(K) for dj in range(K)]
# DVE-solo taps need even dj (4B alignment for packed bf16 reads).
DVE_TAPS = [(di, dj) for (di, dj) in ALL_TAPS if dj in (0, 2, 4)][:6]
ACT_TAPS = [t for t in ALL_TAPS if t not in DVE_TAPS][:12]
PE_TAPS = [t for t in ALL_TAPS if t not in DVE_TAPS and t not in ACT_TAPS]
N_PE = len(PE_TAPS)
DVE_IMPL = "tstt"            # 'tstt' (ts_mul + tt_add) or 'ata'

# blob element offsets (bf16 elements)
SRCH = 0                     # [2, 31, 32]
SRCH_E = 2 * X * XP
TCOL = SRCH_E                # [2, 64] fp32 -> occupies 2*64*2 bf16 slots
TCOL_E = 2 * 64 * 2
DIAG = SRCH + SRCH_E + TCOL_E  # [2, N_PE, 128]
SECT_E = DIAG + 2 * N_PE * 128

bf16 = mybir.dt.bfloat16
f32 = mybir.dt.float32

_CACHE = {}


def _split_excess_waits(nc):
    """Walrus codegen allows a single sync-wait command per instruction.
    Move extra waits onto inserted same-engine NoOps; firing a monotone
    wait earlier on the same queue is always safe."""
    for fn in nc.m.functions:
        for bb in fn.blocks:
            out = []
            for inst in bb.instructions:
                si = inst.sync_info
                if si is not None and len(si.on_wait) > 1:
                    waits = list(si.on_wait)
                    for w in waits[:-1]:
                        nop = mybir.InstNoOp(
                            name=nc.get_next_instruction_name(), ins=[], outs=[])
                        nop.engine = inst.engine
                        nop.sync_info = mybir.SyncInfo(on_wait=[w], on_update=[])
                        out.append(nop)
                    si.on_wait = [waits[-1]]
                out.append(inst)
            bb.instructions = out


def _sample_views(blob):
    """blob: [C, SECT_E] bf16 tile -> (search, tcolf, diag) views."""
    sv = blob[:, SRCH:SRCH + SRCH_E].rearrange(
        "c (h r j) -> c h r j", h=2, j=XP)
    tc = blob[:, TCOL:TCOL + TCOL_E].bitcast(f32).rearrange(
        "c (h k) -> c h k", h=2)
    dg = blob[:, DIAG:].rearrange("c (h k m) -> c h k m", h=2, m=128)
    return sv, tc, dg


def _emit_sample_compute(nc, sv, tc, dg, acc, tmpp, psums):
    """Emit PE taps + ACT/DVE taps for one sample. psums[(h, blk)] tiles."""
    ADD = mybir.AluOpType.add
    # PE: diag matmuls, taps outer so both blocks reuse the loaded stationary
    for h in range(2):
        for ki, (di, dj) in enumerate(PE_TAPS):
            for blk, (r_base, nrows) in enumerate(((0, R0), (R0, R1))):
                rows = sv[:, h, r_base + di:r_base + di + nrows, dj:dj + OUT]
                nc.tensor.matmul(psums[(h, blk)][:, :, :], dg[:, h, ki, :],
                                 rows, start=(ki == 0), stop=False,
                                 skip_group_check=True)

    def win_t(di, dj, h):
        return (sv[:, h, di:di + OUT, dj:dj + 26],
                tc[:, h, _tap_idx(di, dj):_tap_idx(di, dj) + 1])

    # Interleave ACT-paired and DVE-solo taps so both engines stream.
    order = []
    na, nd = len(ACT_TAPS), len(DVE_TAPS)
    ia = id_ = 0
    for i in range(na + nd):
        take_pair = (ia * max(nd, 1) <= id_ * max(na, 1)) if nd else True
        if ia < na and (take_pair or id_ >= nd):
            order.append(("pair", ACT_TAPS[ia])); ia += 1
        else:
            order.append(("solo", DVE_TAPS[id_])); id_ += 1
    first = True
    for kind, (di, dj) in order:
        if kind == "solo":
            for h in range(2):
                win, t1 = win_t(di, dj, h)
                if first:
                    nc.vector.tensor_scalar_mul(acc[:, h], win, t1)
                else:
                    tmph = tmpp.tile([C, OUT, 26], bf16, tag=f"tmp{h}",
                                     name=f"tmp{h}")
                    nc.vector.tensor_scalar_mul(tmph[:], win, t1)
                    nc.vector.tensor_tensor(out=acc[:, h], in0=tmph[:],
                                            in1=acc[:, h], op=ADD)
            first = False
        else:
            tmp2 = tmpp.tile([C, 2, OUT, 26], bf16, tag="tmp2", name="tmp2")
            for h in range(2):
                win, t1 = win_t(di, dj, h)
                if first:
                    nc.scalar.mul(acc[:, h], win, t1)
                else:
                    nc.scalar.mul(tmp2[:, h], win, t1)
            if not first:
                nc.vector.tensor_tensor(out=acc[:], in0=tmp2[:], in1=acc[:],
                                        op=ADD)
            first = False


def _tap_idx(di, dj):
    return di * K + dj


def _emit_fold_and_out(nc, ident, acc, psums, out_sb, o_out, s):
    """Fold acc into psum via identity matmul, evacuate with ACT, DMA out."""
    for h in range(2):
        for blk, (r_base, nrows) in enumerate(((0, R0), (R0, R1))):
            pt = psums[(h, blk)]
            nc.tensor.matmul(pt[:, :, :], ident[:, :],
                             acc[:, h, r_base:r_base + nrows, 0:OUT],
                             start=False, stop=True, skip_group_check=True)
            nc.scalar.copy(out_sb[:, h, r_base:r_base + nrows, :],
                           pt[:, :, :])
    nc.sync.dma_start(out=o_out[s], in_=out_sb[:])


def _build_nc(reps=1):
    nc = bass.Bass("TRN2", debug=False)
    b_in = nc.dram_tensor("blob", [BPC, C, SECT_E], bf16,
                          kind="ExternalInput").ap()
    id_in = nc.dram_tensor("ident", [C, 128], bf16, kind="ExternalInput").ap()
    o_out = nc.dram_tensor("o", [BPC, C, 2, OUT, OUT], f32,
                           kind="ExternalOutput").ap()
    with tile.TileContext(nc) as tc:
        with tc.tile_pool(name="sb", bufs=3) as sb, \
             tc.tile_pool(name="work", bufs=3) as work, \
             tc.tile_pool(name="tmpp", bufs=2) as tmpp, \
             tc.tile_pool(name="outp", bufs=2) as outp, \
             tc.tile_pool(name="con", bufs=1) as con, \
             tc.tile_pool(name="ps", bufs=2, space="PSUM") as ps:
            ident = con.tile([C, 128], bf16, tag="ident")
            nc.sync.dma_start(out=ident[:], in_=id_in)
            for _ in range(reps):
                pending = None
                for s in range(BPC):
                    blob = sb.tile([C, SECT_E], bf16, tag="blob")
                    nc.sync.dma_start(out=blob[:], in_=b_in[s])
                    sv, tcv, dgv = _sample_views(blob)
                    acc = work.tile([C, 2, OUT, 26], bf16, tag="acc")
                    tmp = tmpp.tile([C, 2, OUT, 26], bf16, tag="tmp")
                    psums = {}
                    for h in range(2):
                        for blk, nrows in ((0, R0), (1, R1)):
                            ptile = ps.tile([C, nrows, OUT], f32,
                                            tag=f"p{h}{blk}", name=f"p{h}{blk}")
                            psums[(h, blk)] = ptile
                    _emit_sample_compute(nc, sv, tcv, dgv, acc, tmp, psums)
                    if pending is not None:
                        _emit_fold_and_out(nc, ident, *pending)
                    out_sb = outp.tile([C, 2, OUT, OUT], f32, tag="out_sb")
                    pending = (acc, psums, out_sb, o_out, s)
                _emit_fold_and_out(nc, ident, *pending)
    _split_excess_waits(nc)
    return nc


def _marshal(search, template):
    """-> blob [B, C, SECT_E] bf16 (raw bytes hold mixed bf16/f32 sections)."""
    search = np.ascontiguousarray(search, dtype=np.float32)
    template = np.ascontiguousarray(template, dtype=np.float32)
    # search -> [B, C, 2, 31, 32] bf16 (channel-half major, padded rows)
    s_cm = search.reshape(B, X, X, 2, C).transpose(0, 4, 3, 1, 2)  # [B,C,2,31,31]
    srch = np.zeros((B, C, 2, X, XP), ml_dtypes.bfloat16)
    srch[:, :, :, :, :X] = s_cm.astype(ml_dtypes.bfloat16)
    # template cols fp32 [B, C, 2, 64]
    t_cm = template.reshape(B, K * K, 2, C).transpose(0, 3, 2, 1)  # [B,C,2,49]
    tcol = np.zeros((B, C, 2, 64), np.float32)
    tcol[:, :, :, :K * K] = t_cm
    # diag tiles bf16 [B, C, 2, N_PE, 128]
    t_bf = t_cm.astype(ml_dtypes.bfloat16)
    diag = np.zeros((B, C, 2, N_PE, 128), ml_dtypes.bfloat16)
    c = np.arange(C)
    pe_idx = np.array([_tap_idx(di, dj) for (di, dj) in PE_TAPS])
    # diag[b, c, h, i, c] = t_bf[b, c, h, pe_idx[i]]
    diag[:, c, :, :, c] = t_bf[:, :, :, pe_idx].transpose(1, 0, 2, 3)
    blob = np.concatenate([
        srch.reshape(B, C, -1).view(np.uint16),
        tcol.reshape(B, C, -1).view(np.uint16).reshape(B, C, -1),
        diag.reshape(B, C, -1).view(np.uint16),
    ], axis=2)
    return blob.view(ml_dtypes.bfloat16)


def _unmarshal(results):
    o = np.stack([results[core]["o"] for core in range(N_CORES)])
    # [cores, BPC, C, 2, OUT, OUT] -> [B, OUT, OUT, 2, C] -> [B, OUT, OUT, CH]
    o = o.reshape(B, C, 2, OUT, OUT).transpose(0, 3, 4, 2, 1).reshape(
        B, OUT, OUT, CH)
    return np.ascontiguousarray(o)


def kernel(search, template):
    if "nc" not in _CACHE:
        _CACHE["nc"] = _build_nc()
    nc = _CACHE["nc"]
    blob = _marshal(search, template).reshape(N_CORES, BPC, C, SECT_E)
    ident = np.eye(C, 128, dtype=ml_dtypes.bfloat16)
    in_maps = [{"blob": blob[core], "ident": ident} for core in range(N_CORES)]
    res = run_bass_kernel_spmd(nc, in_maps, core_ids=list(range(N_CORES)))
    return _unmarshal(res.results)


# revision 9
# speedup vs baseline: 1.3751x; 1.3751x over previous
"""Depthwise cross-correlation (per-sample dynamic kernel) on 8 Trainium2 cores.

reference: out[b,i,j,c] = sum_{di,dj} search[b,i+di,j+dj,c] * template[b,di,dj,c]
  search [64,31,31,256] f32, template [64,7,7,256] f32 -> out [64,25,25,256] f32

Strategy (pure data parallel, 8 samples/core, no collectives), all-bf16 compute:
- 49 taps split across three engine paths, balanced so PE/ACT/DVE finish
  together:
  * PE taps: diag(t_k) matmuls accumulated in PSUM (fp32), bf16 operands at
    1 col/cycle. Output rows split 13+12 so each accumulation target fits one
    PSUM bank.
  * ACT-paired taps: ScalarE per-channel multiply (Copy activation with
    per-partition scale) into a product buffer, VectorE tensor_tensor add
    (bf16 2x mode, both halves fused in one 4D instruction) into acc.
  * DVE-solo taps: single-pass FMA on VectorE.
- acc is folded into PSUM by one extra PE matmul with an identity stationary
  (accumulate), delayed one sample so PE never stalls on DVE.
- ScalarE evacuates the final PSUM to SBUF fp32; DMA out.
- Search rows are padded 31->32 cols so bf16 windows stay 4B-aligned (even dj
  for any DVE-strided read).
- A post-pass splits multi-wait instructions (walrus allows one sync-wait
  per instruction) into single-wait NoOp carriers.
"""
import sys

sys.path.insert(0, "/opt/trn_rl_repo")

import numpy as np
import ml_dtypes
import concourse.bass as bass
import concourse.mybir as mybir
import concourse.tile as tile
from concourse.bass_utils import run_bass_kernel_spmd

B = 64
X, K, OUT = 31, 7, 25
XP = 32                      # padded row length
CH = 256
C = 128                      # channels per half (partition dim)
N_CORES = 8
BPC = B // N_CORES           # samples per core
R0, R1 = 13, 12              # output row split (325 / 300 psum cols)

# ---- tap assignment (tune) -------------------------------------------------
ALL_TAPS = [(di, dj) for di in range(K) for dj in range(K)]
DVE_IMPL = "tstt"            # 'tstt' (ts_mul + tt_add) or 'ata'


def configure(n_act=12, n_dve=6):
    """Set the tap split and recompute blob layout offsets."""
    global DVE_TAPS, ACT_TAPS, PE_TAPS, N_PE
    global SRCH, SRCH_E, TCOL, TCOL_E, DIAG, SECT_E
    # DVE-solo taps need even dj (4B alignment for packed bf16 reads).
    DVE_TAPS = [(di, dj) for (di, dj) in ALL_TAPS if dj in (0, 2, 4)][:n_dve]
    ACT_TAPS = [t for t in ALL_TAPS if t not in DVE_TAPS][:n_act]
    PE_TAPS = [t for t in ALL_TAPS
               if t not in DVE_TAPS and t not in ACT_TAPS]
    N_PE = len(PE_TAPS)
    # blob element offsets (bf16 elements)
    SRCH = 0                     # [2, 31, 32]
    SRCH_E = 2 * X * XP
    TCOL = SRCH_E                # [2, 64] fp32 -> occupies 2*64*2 bf16 slots
    TCOL_E = 2 * 64 * 2
    DIAG = SRCH + SRCH_E + TCOL_E  # [2, N_PE, 128]
    SECT_E = DIAG + 2 * N_PE * 128
    _CACHE.pop("nc", None)


_CACHE = {}
configure()

bf16 = mybir.dt.bfloat16
f32 = mybir.dt.float32


def _split_excess_waits(nc):
    """Walrus codegen allows a single sync-wait command per instruction.
    Move extra waits onto inserted same-engine NoOps; firing a monotone
    wait earlier on the same queue is always safe."""
    for fn in nc.m.functions:
        for bb in fn.blocks:
            out = []
            for inst in bb.instructions:
                si = inst.sync_info
                if si is not None and len(si.on_wait) > 1:
                    waits = list(si.on_wait)
                    for w in waits[:-1]:
                        nop = mybir.InstNoOp(
                            name=nc.get_next_instruction_name(), ins=[], outs=[])
                        nop.engine = inst.engine
                        nop.sync_info = mybir.SyncInfo(on_wait=[w], on_update=[])
                        out.append(nop)
                    si.on_wait = [waits[-1]]
                out.append(inst)
            bb.instructions = out


def _sample_views(blob):
    """blob: [C, SECT_E] bf16 tile -> (search, tcolf, diag) views."""
    sv = blob[:, SRCH:SRCH + SRCH_E].rearrange(
        "c (h r j) -> c h r j", h=2, j=XP)
    tc = blob[:, TCOL:TCOL + TCOL_E].bitcast(f32).rearrange(
        "c (h k) -> c h k", h=2)
    dg = blob[:, DIAG:].rearrange("c (h k m) -> c h k m", h=2, m=128)
    return sv, tc, dg


def _emit_sample_compute(nc, sv, tc, dg, acc, tmpp, psums):
    """Emit PE taps + ACT/DVE taps for one sample. psums[(h, blk)] tiles."""
    ADD = mybir.AluOpType.add
    # PE: diag matmuls, taps outer so both blocks reuse the loaded stationary
    for h in range(2):
        for ki, (di, dj) in enumerate(PE_TAPS):
            for blk, (r_base, nrows) in enumerate(((0, R0), (R0, R1))):
                rows = sv[:, h, r_base + di:r_base + di + nrows, dj:dj + OUT]
                nc.tensor.matmul(psums[(h, blk)][:, :, :], dg[:, h, ki, :],
                                 rows, start=(ki == 0), stop=False,
                                 skip_group_check=True)

    def win_t(di, dj, h):
        return (sv[:, h, di:di + OUT, dj:dj + 26],
                tc[:, h, _tap_idx(di, dj):_tap_idx(di, dj) + 1])

    # Interleave ACT-paired and DVE-solo taps so both engines stream.
    order = []
    na, nd = len(ACT_TAPS), len(DVE_TAPS)
    ia = id_ = 0
    for i in range(na + nd):
        take_pair = (ia * max(nd, 1) <= id_ * max(na, 1)) if nd else True
        if ia < na and (take_pair or id_ >= nd):
            order.append(("pair", ACT_TAPS[ia])); ia += 1
        else:
            order.append(("solo", DVE_TAPS[id_])); id_ += 1
    first = True
    for kind, (di, dj) in order:
        if kind == "solo":
            for h in range(2):
                win, t1 = win_t(di, dj, h)
                if first:
                    nc.vector.tensor_scalar_mul(acc[:, h], win, t1)
                else:
                    tmph = tmpp.tile([C, OUT, 26], bf16, tag=f"tmp{h}",
                                     name=f"tmp{h}")
                    nc.vector.tensor_scalar_mul(tmph[:], win, t1)
                    nc.vector.tensor_tensor(out=acc[:, h], in0=tmph[:],
                                            in1=acc[:, h], op=ADD)
            first = False
        else:
            tmp2 = tmpp.tile([C, 2, OUT, 26], bf16, tag="tmp2", name="tmp2")
            for h in range(2):
                win, t1 = win_t(di, dj, h)
                if first:
                    nc.scalar.mul(acc[:, h], win, t1)
                else:
                    nc.scalar.mul(tmp2[:, h], win, t1)
            if not first:
                nc.vector.tensor_tensor(out=acc[:], in0=tmp2[:], in1=acc[:],
                                        op=ADD)
            first = False


def _tap_idx(di, dj):
    return di * K + dj


def _emit_fold_and_out(nc, ident, acc, psums, out_sb, o_out, s):
    """Fold acc into psum via identity matmul, evacuate with ACT, DMA out."""
    for h in range(2):
        for blk, (r_base, nrows) in enumerate(((0, R0), (R0, R1))):
            pt = psums[(h, blk)]
            nc.tensor.matmul(pt[:, :, :], ident[:, :],
                             acc[:, h, r_base:r_base + nrows, 0:OUT],
                             start=False, stop=True, skip_group_check=True)
            nc.scalar.copy(out_sb[:, h, r_base:r_base + nrows, :],
                           pt[:, :, :])
    nc.sync.dma_start(out=o_out[s], in_=out_sb[:])


def _build_nc(reps=1):
    nc = bass.Bass("TRN2", debug=False)
    b_in = nc.dram_tensor("blob", [BPC, C, SECT_E], bf16,
                          kind="ExternalInput").ap()
    id_in = nc.dram_tensor("ident", [C, 128], bf16, kind="ExternalInput").ap()
    o_out = nc.dram_tensor("o", [BPC, C, 2, OUT, OUT], f32,
                           kind="ExternalOutput").ap()
    with tile.TileContext(nc) as tc:
        with tc.tile_pool(name="sb", bufs=3) as sb, \
             tc.tile_pool(name="work", bufs=3) as work, \
             tc.tile_pool(name="tmpp", bufs=2) as tmpp, \
             tc.tile_pool(name="outp", bufs=2) as outp, \
             tc.tile_pool(name="con", bufs=1) as con, \
             tc.tile_pool(name="ps", bufs=2, space="PSUM") as ps:
            ident = con.tile([C, 128], bf16, tag="ident")
            nc.sync.dma_start(out=ident[:], in_=id_in)
            for _ in range(reps):
                pending = None
                for s in range(BPC):
                    blob = sb.tile([C, SECT_E], bf16, tag="blob")
                    nc.sync.dma_start(out=blob[:], in_=b_in[s])
                    sv, tcv, dgv = _sample_views(blob)
                    acc = work.tile([C, 2, OUT, 26], bf16, tag="acc")
                    psums = {}
                    for h in range(2):
                        for blk, nrows in ((0, R0), (1, R1)):
                            ptile = ps.tile([C, nrows, OUT], f32,
                                            tag=f"p{h}{blk}", name=f"p{h}{blk}")
                            psums[(h, blk)] = ptile
                    _emit_sample_compute(nc, sv, tcv, dgv, acc, tmpp, psums)
                    if pending is not None:
                        _emit_fold_and_out(nc, ident, *pending)
                    out_sb = outp.tile([C, 2, OUT, OUT], f32, tag="out_sb")
                    pending = (acc, psums, out_sb, o_out, s)
                _emit_fold_and_out(nc, ident, *pending)
    _split_excess_waits(nc)
    return nc


def _marshal(search, template):
    """-> blob [B, C, SECT_E] bf16 (raw bytes hold mixed bf16/f32 sections)."""
    search = np.ascontiguousarray(search, dtype=np.float32)
    template = np.ascontiguousarray(template, dtype=np.float32)
    # search -> [B, C, 2, 31, 32] bf16 (channel-half major, padded rows)
    s_cm = search.reshape(B, X, X, 2, C).transpose(0, 4, 3, 1, 2)  # [B,C,2,31,31]
    srch = np.zeros((B, C, 2, X, XP), ml_dtypes.bfloat16)
    srch[:, :, :, :, :X] = s_cm.astype(ml_dtypes.bfloat16)
    # template cols fp32 [B, C, 2, 64]
    t_cm = template.reshape(B, K * K, 2, C).transpose(0, 3, 2, 1)  # [B,C,2,49]
    tcol = np.zeros((B, C, 2, 64), np.float32)
    tcol[:, :, :, :K * K] = t_cm
    # diag tiles bf16 [B, C, 2, N_PE, 128]
    t_bf = t_cm.astype(ml_dtypes.bfloat16)
    diag = np.zeros((B, C, 2, N_PE, 128), ml_dtypes.bfloat16)
    c = np.arange(C)
    pe_idx = np.array([_tap_idx(di, dj) for (di, dj) in PE_TAPS])
    # diag[b, c, h, i, c] = t_bf[b, c, h, pe_idx[i]]
    diag[:, c, :, :, c] = t_bf[:, :, :, pe_idx].transpose(1, 0, 2, 3)
    blob = np.concatenate([
        srch.reshape(B, C, -1).view(np.uint16),
        tcol.reshape(B, C, -1).view(np.uint16).reshape(B, C, -1),
        diag.reshape(B, C, -1).view(np.uint16),
    ], axis=2)
    return blob.view(ml_dtypes.bfloat16)


def _unmarshal(results):
    o = np.stack([results[core]["o"] for core in range(N_CORES)])
    # [cores, BPC, C, 2, OUT, OUT] -> [B, OUT, OUT, 2, C] -> [B, OUT, OUT, CH]
    o = o.reshape(B, C, 2, OUT, OUT).transpose(0, 3, 4, 2, 1).reshape(
        B, OUT, OUT, CH)
    return np.ascontiguousarray(o)


def kernel(search, template):
    if "nc" not in _CACHE:
        _CACHE["nc"] = _build_nc()
    nc = _CACHE["nc"]
    blob = _marshal(search, template).reshape(N_CORES, BPC, C, SECT_E)
    ident = np.eye(C, 128, dtype=ml_dtypes.bfloat16)
    in_maps = [{"blob": blob[core], "ident": ident} for core in range(N_CORES)]
    res = run_bass_kernel_spmd(nc, in_maps, core_ids=list(range(N_CORES)))
    return _unmarshal(res.results)


# revision 18
# speedup vs baseline: 1.4287x; 1.0390x over previous
"""Depthwise cross-correlation (per-sample dynamic kernel) on 8 Trainium2 cores.

reference: out[b,i,j,c] = sum_{di,dj} search[b,i+di,j+dj,c] * template[b,di,dj,c]
  search [64,31,31,256] f32, template [64,7,7,256] f32 -> out [64,25,25,256] f32

Strategy (pure data parallel, 8 samples/core, no collectives), all-bf16 compute.
The 64x31x31x256 job is split into (sample, channel-half) units: 16 units/core,
each [128 partitions x 31x31] with its own DMA; units pipeline independently.
Per unit the 49 taps are split across three engine paths, balanced to finish
together:
  * PE taps: diag(t_k) matmuls accumulated in PSUM (fp32), bf16 at 1 col/cyc;
    output rows split 13+12 to fit PSUM banks; redundant LDWEIGHTS are deduped
    by a post-pass (the stationary persists across the two row-block matmuls).
  * ACT-paired taps: ScalarE per-channel multiply (Copy activation with
    per-partition scale AP) into a rotating product buffer; VectorE
    tensor_tensor add (bf16 2x mode) into acc.
  * DVE-solo taps: tensor_scalar mul (bf16 4x mode) + tensor_tensor add.
    Solo taps use even dj so strided window reads stay 4B-aligned.
- acc folds into PSUM via one extra PE matmul with an identity stationary
  (accumulate); ScalarE evacuates PSUM to SBUF fp32; DMA out.
- Search rows padded 31->32 cols for alignment.
- Post-passes: LDWEIGHTS dedupe, then multi-wait splitting (walrus allows one
  sync-wait per instruction).
"""
import sys

sys.path.insert(0, "/opt/trn_rl_repo")

import numpy as np
import ml_dtypes
import concourse.bass as bass
import concourse.mybir as mybir
import concourse.tile as tile
from concourse.bass_utils import run_bass_kernel_spmd

B = 64
X, K, OUT = 31, 7, 25
XP = 32                      # padded row length
CH = 256
C = 128                      # channels per half (partition dim)
N_CORES = 8
BPC = B // N_CORES           # samples per core
NU = 2 * BPC                 # units per core (sample, half)
R0, R1 = 13, 12              # output row split (325 / 300 psum cols)

ALL_TAPS = [(di, dj) for di in range(K) for dj in range(K)]

bf16 = mybir.dt.bfloat16
f32 = mybir.dt.float32

_CACHE = {}


def _mk_split(n_act, n_dve):
    dve = [(di, dj) for (di, dj) in ALL_TAPS if dj in (0, 2, 4)][:n_dve]
    act = [t for t in ALL_TAPS if t not in dve][:n_act]
    pe = [t for t in ALL_TAPS if t not in dve and t not in act]
    return pe, act, dve


def configure(n_act=10, n_dve=8, last_act=None, last_dve=None):
    """Set tap split (optionally different for the final units) and layout."""
    global SPLITS, N_PE, SRCH, TCOL, DIAG, USECT
    main = _mk_split(n_act, n_dve)
    last = _mk_split(last_act, last_dve) if last_act is not None else main
    SPLITS = [main] * (NU - 2) + [last, last]
    N_PE = max(len(s[0]) for s in SPLITS)
    # per-unit blob element offsets (bf16 elements)
    SRCH = 0                       # [31, 32] bf16
    TCOL = X * XP                  # [64] f32 (2 bf16 slots each)
    DIAG = TCOL + 64 * 2           # [N_PE, 128] bf16
    USECT = DIAG + N_PE * 128
    _CACHE.pop("nc", None)


def _tap_idx(di, dj):
    return di * K + dj


def _ap_key(arg):
    try:
        return repr(arg)
    except Exception:
        return None


def _dedupe_ldweights(nc):
    """Drop an InstLdweights whose weights AP is identical to the previous
    weight load on the PE stream (the stationary persists across matmuls).
    Waits/updates of dropped loads migrate to the next kept PE instruction."""
    for fn in nc.m.functions:
        for bb in fn.blocks:
            out = []
            prev_key = None
            pe_engine = None
            pend_w, pend_u = [], []
            for inst in bb.instructions:
                if isinstance(inst, mybir.InstLdweights):
                    pe_engine = inst.engine
                    key = _ap_key(inst.ins[0])
                    if key is not None and key == prev_key:
                        si = inst.sync_info
                        if si is not None:
                            pend_w.extend(si.on_wait)
                            pend_u.extend(si.on_update)
                        continue
                    prev_key = key
                if (pend_w or pend_u) and inst.engine == pe_engine:
                    si = inst.sync_info
                    if si is None:
                        inst.sync_info = mybir.SyncInfo(
                            on_wait=list(pend_w), on_update=list(pend_u))
                    else:
                        si.on_wait = list(si.on_wait) + pend_w
                        si.on_update = list(si.on_update) + pend_u
                    pend_w, pend_u = [], []
                out.append(inst)
            assert not (pend_w or pend_u), "dangling sync from dropped LDW"
            bb.instructions = out


def _split_excess_waits(nc):
    """Walrus codegen allows a single sync-wait command per instruction.
    Move extra waits onto inserted same-engine NoOps; firing a monotone
    wait earlier on the same queue is always safe."""
    for fn in nc.m.functions:
        for bb in fn.blocks:
            out = []
            for inst in bb.instructions:
                si = inst.sync_info
                if si is not None and len(si.on_wait) > 1:
                    waits = list(si.on_wait)
                    for w in waits[:-1]:
                        nop = mybir.InstNoOp(
                            name=nc.get_next_instruction_name(), ins=[], outs=[])
                        nop.engine = inst.engine
                        nop.sync_info = mybir.SyncInfo(on_wait=[w], on_update=[])
                        out.append(nop)
                    si.on_wait = [waits[-1]]
                out.append(inst)
            bb.instructions = out


def _emit_unit(nc, pools, ident, b_in, o_out, u, split):
    sb, work, tmpp, outp, ps = pools
    pe_taps, act_taps, dve_taps = split
    ADD = mybir.AluOpType.add
    blob = sb.tile([C, USECT], bf16, tag="blob")
    nc.sync.dma_start(out=blob[:], in_=b_in[u])
    sv = blob[:, SRCH:SRCH + X * XP].rearrange("c (r j) -> c r j", j=XP)
    tc = blob[:, TCOL:TCOL + 128].bitcast(f32)
    dg = blob[:, DIAG:DIAG + len(pe_taps) * 128].rearrange(
        "c (k m) -> c k m", m=128)
    acc = work.tile([C, OUT, 26], bf16, tag="acc")
    pa = ps.tile([C, R0, OUT], f32, tag="pa")
    pb = ps.tile([C, R1, OUT], f32, tag="pb")
    # PE taps
    for ki, (di, dj) in enumerate(pe_taps):
        for pt, r_base, nrows in ((pa, 0, R0), (pb, R0, R1)):
            rows = sv[:, r_base + di:r_base + di + nrows, dj:dj + OUT]
            nc.tensor.matmul(pt[:, :, :], dg[:, ki, :], rows,
                             start=(ki == 0), stop=False,
                             skip_group_check=True)

    # interleave ACT-paired and DVE-solo taps so both engines stream
    order = []
    na, nd = len(act_taps), len(dve_taps)
    ia = idv = 0
    for _ in range(na + nd):
        take_pair = (ia * max(nd, 1) <= idv * max(na, 1)) if nd else True
        if ia < na and (take_pair or idv >= nd):
            order.append(("pair", act_taps[ia])); ia += 1
        else:
            order.append(("solo", dve_taps[idv])); idv += 1
    first = True
    for kind, (di, dj) in order:
        win = sv[:, di:di + OUT, dj:dj + 26]
        t1 = tc[:, _tap_idx(di, dj):_tap_idx(di, dj) + 1]
        if kind == "solo":
            if first:
                nc.vector.tensor_scalar_mul(acc[:], win, t1)
            else:
                tmph = tmpp.tile([C, OUT, 26], bf16, tag="tmp", name="tmp")
                nc.vector.tensor_scalar_mul(tmph[:], win, t1)
                nc.vector.tensor_tensor(out=acc[:], in0=tmph[:], in1=acc[:],
                                        op=ADD)
        else:
            if first:
                nc.scalar.mul(acc[:], win, t1)
            else:
                tmph = tmpp.tile([C, OUT, 26], bf16, tag="tmp", name="tmp")
                nc.scalar.mul(tmph[:], win, t1)
                nc.vector.tensor_tensor(out=acc[:], in0=tmph[:], in1=acc[:],
                                        op=ADD)
        first = False

    out_sb = outp.tile([C, OUT, OUT], f32, tag="out_sb")
    if order:
        # fold acc into psum (accumulate), then evacuate fp32
        for pt, r_base, nrows in ((pa, 0, R0), (pb, R0, R1)):
            nc.tensor.matmul(pt[:, :, :], ident[:, :],
                             acc[:, r_base:r_base + nrows, 0:OUT],
                             start=False, stop=True, skip_group_check=True)
            nc.scalar.copy(out_sb[:, r_base:r_base + nrows, :], pt[:, :, :])
    else:
        for pt, r_base, nrows in ((pa, 0, R0), (pb, R0, R1)):
            nc.scalar.copy(out_sb[:, r_base:r_base + nrows, :], pt[:, :, :])
    nc.sync.dma_start(out=o_out[u], in_=out_sb[:])


def _build_nc(reps=1):
    nc = bass.Bass("TRN2", debug=False)
    b_in = nc.dram_tensor("blob", [NU, C, USECT], bf16,
                          kind="ExternalInput").ap()
    id_in = nc.dram_tensor("ident", [C, 128], bf16, kind="ExternalInput").ap()
    o_out = nc.dram_tensor("o", [NU, C, OUT, OUT], f32,
                           kind="ExternalOutput").ap()
    with tile.TileContext(nc) as tc:
        with tc.tile_pool(name="sb", bufs=3) as sb, \
             tc.tile_pool(name="work", bufs=3) as work, \
             tc.tile_pool(name="tmpp", bufs=3) as tmpp, \
             tc.tile_pool(name="outp", bufs=3) as outp, \
             tc.tile_pool(name="con", bufs=1) as con, \
             tc.tile_pool(name="ps", bufs=3, space="PSUM") as ps:
            ident = con.tile([C, 128], bf16, tag="ident")
            nc.sync.dma_start(out=ident[:], in_=id_in)
            pools = (sb, work, tmpp, outp, ps)
            for _ in range(reps):
                for u in range(NU):
                    _emit_unit(nc, pools, ident, b_in, o_out, u, SPLITS[u])
    _dedupe_ldweights(nc)
    _split_excess_waits(nc)
    return nc


def _marshal(search, template):
    """-> blob [B, 2, C, USECT] bf16 (bytes hold mixed bf16/f32 sections)."""
    search = np.ascontiguousarray(search, dtype=np.float32)
    template = np.ascontiguousarray(template, dtype=np.float32)
    # search -> [B, 2, C, 31, 32] bf16 (channel-half major, padded rows)
    s_cm = search.reshape(B, X, X, 2, C).transpose(0, 3, 4, 1, 2)
    srch = np.zeros((B, 2, C, X, XP), ml_dtypes.bfloat16)
    srch[:, :, :, :, :X] = s_cm.astype(ml_dtypes.bfloat16)
    # template cols fp32 [B, 2, C, 64]
    t_cm = template.reshape(B, K * K, 2, C).transpose(0, 2, 3, 1)  # [B,2,C,49]
    tcol = np.zeros((B, 2, C, 64), np.float32)
    tcol[:, :, :, :K * K] = t_cm
    # diag tiles bf16 [B, 2, C, N_PE, 128]; unused tap slots stay zero
    t_bf = t_cm.astype(ml_dtypes.bfloat16)
    diag = np.zeros((B, 2, C, N_PE, 128), ml_dtypes.bfloat16)
    c = np.arange(C)
    for u in range(NU):
        pe_taps = SPLITS[u][0]
        s, h = divmod(u, 2)
        idx = np.array([_tap_idx(di, dj) for (di, dj) in pe_taps])
        bs = np.arange(s, B, BPC)  # batch entries at per-core sample slot s
        # diag[b, h, c, i, c] = t_bf[b, h, c, idx[i]]
        diag[bs[:, None, None], h, c[:, None], np.arange(len(idx))[None, :],
             c[:, None]] = t_bf[bs[:, None, None], h, c[:, None], idx[None, :]]
    blob = np.concatenate([
        srch.reshape(B, 2, C, -1).view(np.uint16),
        tcol.reshape(B, 2, C, -1).view(np.uint16).reshape(B, 2, C, -1),
        diag.reshape(B, 2, C, -1).view(np.uint16),
    ], axis=3)
    return blob.view(ml_dtypes.bfloat16)


def _unmarshal(results):
    o = np.stack([results[core]["o"] for core in range(N_CORES)])
    # [cores, NU, C, OUT, OUT]; NU = (sample, half)
    o = o.reshape(B, 2, C, OUT, OUT).transpose(0, 3, 4, 1, 2).reshape(
        B, OUT, OUT, CH)
    return np.ascontiguousarray(o)


def kernel(search, template):
    if "nc" not in _CACHE:
        _CACHE["nc"] = _build_nc()
    nc = _CACHE["nc"]
    blob = _marshal(search, template).reshape(N_CORES, NU, C, USECT)
    ident = np.eye(C, 128, dtype=ml_dtypes.bfloat16)
    in_maps = [{"blob": blob[core], "ident": ident} for core in range(N_CORES)]
    res = run_bass_kernel_spmd(nc, in_maps, core_ids=list(range(N_CORES)))
    return _unmarshal(res.results)


configure()
